# revision 39
# baseline (speedup 1.0000x reference)
"""Trainium2 Bass kernel for a GPT-style transformer block (B=2, T=2048, C=768,
NH=12, HD=64, DFF=3072), distributed over 8 NeuronCores.

Sharding: token-data-parallel with zigzag strip assignment, zero collectives.
  - cores 0-3 process batch 0, cores 4-7 batch 1.
  - within a batch, rank r owns token strips r and 7-r (strips of 256 tokens).
  - each core redundantly computes K/V for tokens [0, 256*(8-r)) (its causal
    prefix), so no cross-core communication is needed at all.
4 distinct per-rank programs are compiled and dispatched concurrently to the 8
devices via async PJRT.

v2: all matmul operands in bf16 (weights pre-cast on the host; activations
rounded during LN/eviction).  LN1 + K/V/Q GEMMs fused into one streaming sweep
over 256-token tiles to keep the PE warm.  Attention processes head PAIRS so
the two K=64 QK matmuls row-tile concurrently in the PE array.  K/Q PSUM
evictions ride the Scalar engine (Copy+bias); V/c_proj/proj biases ride K=1
ones-matmuls into the PSUM accumulation.  Softmax stays exp-without-max with
the denominator as a 65th ones-row of V.

v3 (this version):
  - K/V/Q and c_proj GEMMs in fp8e4 with DoubleRow perf mode (~1.7x PE):
    weights pre-scaled (wk/wv x512, wq x4096, wcp x1) + pair-interleaved on
    the host into [128, 3, 2, C]; descale folded into the PSUM evictions.
  - stage A software-pipelined one 256-token block: iteration i does
    LN1/transpose/fp8-copy for block i and the K/V/Q GEMMs + evictions for
    block i-1, so the per-engine in-order streams never handshake within a
    block.  hT copies ride ACT, V evictions ride DVE (engine balancing).
  - ~60 dummy transposes at t=0 warm the HAM clock gate (PE 1.2->2.4 GHz)
    under the initial DMAs; more dummies bridge the attention->MLP gap.
  - c_proj is interleaved into attention: partial products per head-pair-pair
    (fp8 DR) accumulate into x1_sb via DVE adds, issued one pair late so the
    PE stream never blocks on a fresh softmax normalize.
  - LN2 apply rides ACT (scale=rstd, bias=-mu*rstd APs); proj outputs are
    evicted + stored per m-tile as soon as each accumulation group closes.
"""

import sys
import types
import functools

sys.path.insert(0, "/opt/trn_rl_repo")

# ---- antenv.axon_hooks shim (missing module in this image) -----------------
if "antenv.axon_hooks" not in sys.modules:
    _hooks = types.ModuleType("antenv.axon_hooks")
    _hooks._hook = None
    _hooks.set_axon_ntff_profile_hook = lambda h: setattr(_hooks, "_hook", h)
    _hooks.get_axon_ntff_profile_hook = lambda: _hooks._hook
    sys.modules["antenv.axon_hooks"] = _hooks
    try:
        import antenv

        antenv.axon_hooks = _hooks
    except ImportError:
        pass

import numpy as np
import jax

import concourse.bass as bass
import concourse.mybir as mybir
import concourse.tile as tile
from concourse import bacc
from concourse.bass2jax import (
    _bass_exec_p,
    install_neuronx_cc_hook,
    partition_id_tensor,
)
from concourse.masks import make_identity

B, T, C = 2, 2048, 768
NH, HD, DFF = 12, 64, 64 * 48  # DFF = 3072
STRIP = 256
F32 = mybir.dt.float32
BF16 = mybir.dt.bfloat16
FP8 = mybir.dt.float8e4
EPS = 1e-5
AF = mybir.ActivationFunctionType
DR = mybir.MatmulPerfMode.DoubleRow
SKV = 512.0   # fp8 pre-scale for wk/wv (absmax ~0.17 -> ~87, < 240)
SQ = 4096.0   # fp8 pre-scale for wq (1/8 folded; absmax ~0.021 -> ~87)


# ---------------------------------------------------------------------------
# Per-rank program builder
# ---------------------------------------------------------------------------
def build_rank_program(r: int, use_bias: bool = False):
    """Program for rank r (strips r and 7-r of one batch element)."""
    nc = bacc.Bacc("TRN2", target_bir_lowering=False, debug=False, num_devices=1)

    x_in = nc.declare_dram_parameter("x", [T, C], F32, isOutput=False)
    # qkv weights pre-scaled, fp8, DoubleRow-interleaved on host:
    # w[k, g, o, n] = w_orig[256*g + 128*o + k, n]
    wq_in = nc.declare_dram_parameter("wq", [128, 3, 2, C], FP8, isOutput=False)
    wk_in = nc.declare_dram_parameter("wk", [128, 3, 2, C], FP8, isOutput=False)
    wv_in = nc.declare_dram_parameter("wv", [128, 3, 2, C], FP8, isOutput=False)
    bq_in = nc.declare_dram_parameter("bq", [C], F32, isOutput=False)
    bk_in = nc.declare_dram_parameter("bk", [C], F32, isOutput=False)
    bv_in = nc.declare_dram_parameter("bv", [C], BF16, isOutput=False)
    wcp_in = nc.declare_dram_parameter("wcp", [128, 3, 2, C], FP8, isOutput=False)
    bcp_in = nc.declare_dram_parameter("bcp", [C], BF16, isOutput=False)
    wfc_in = nc.declare_dram_parameter("wfc", [C, DFF], BF16, isOutput=False)
    bfc_in = nc.declare_dram_parameter("bfc", [DFF], F32, isOutput=False)
    wpj_in = nc.declare_dram_parameter("wpj", [DFF, C], BF16, isOutput=False)
    bpj_in = nc.declare_dram_parameter("bpj", [C], BF16, isOutput=False)
    out_dram = nc.declare_dram_parameter("out", [512, C], F32, isOutput=True)

    with tile.TileContext(nc) as tc:
        _build_body(nc, tc, r, use_bias,
                    x_in, wq_in, wk_in, wv_in, bq_in, bk_in, bv_in,
                    wcp_in, bcp_in, wfc_in, bfc_in, wpj_in, bpj_in, out_dram)
    nc.compile()
    return nc


def _build_body(nc, tc, r, use_bias,
                x_in, wq_in, wk_in, wv_in, bq_in, bk_in, bv_in,
                wcp_in, bcp_in, wfc_in, bfc_in, wpj_in, bpj_in, out_dram):
    from contextlib import ExitStack

    sA, sB = r, 7 - r
    NB = 8 - r                 # 256-token tiles in the causal prefix
    NTK = 2 * NB               # 128-token kt chunks in the prefix
    T_kv = NTK * 128

    with ExitStack() as ctx:
        wcp_pool = ctx.enter_context(tc.tile_pool(name="wcp", bufs=1))
        const = ctx.enter_context(tc.tile_pool(name="const", bufs=1))

        # ------- activations spanning stages ---------------------------------
        acts = ctx.enter_context(tc.tile_pool(name="acts", bufs=1))
        yT_sb = acts.tile([128, 6, 512], FP8)        # attn out cols x own q

        sAB = ExitStack()
        actsAB = sAB.enter_context(tc.tile_pool(name="actsAB", bufs=1))
        kT_sb = actsAB.tile([128, 6, T_kv], BF16)    # head-pair rows x keys
        v_sb = actsAB.tile([128, NTK, 12, 65], BF16)
        qT_sb = actsAB.tile([128, 6, 512], BF16)     # head-pair rows x own q

        # =========== stage A: fused LN1 + transpose + K/V/Q GEMMs ===========
        sA_scope = ExitStack()
        xp = sA_scope.enter_context(tc.tile_pool(name="xs", bufs=4))
        wp = sA_scope.enter_context(tc.tile_pool(name="wqkv", bufs=1))
        hT_pool = sA_scope.enter_context(tc.tile_pool(name="hT", bufs=1))
        hT_sb = hT_pool.tile([128, 6, T_kv], FP8)    # ln1(x) transposed, fp8
        ln_pool = sA_scope.enter_context(tc.tile_pool(name="ln", bufs=4))
        tp_ps = sA_scope.enter_context(tc.tile_pool(name="tp_ps", bufs=2, space="PSUM"))
        kq_ps = sA_scope.enter_context(tc.tile_pool(name="kq_ps", bufs=2, space="PSUM"))
        v_ps = sA_scope.enter_context(tc.tile_pool(name="v_ps", bufs=2, space="PSUM"))

        # x streaming: issue the first DMAs before anything else
        x_tiles = {}

        def load_x(b2, split=False):
            t = xp.tile([128, 2, C], F32, tag="x")
            if split:
                for tt in range(2):
                    nc.sync.dma_start(
                        out=t[:, tt, :],
                        in_=x_in[b2 * 256 + tt * 128:b2 * 256 + (tt + 1) * 128, :])
            else:
                nc.sync.dma_start(
                    out=t[:],
                    in_=x_in[b2 * 256:(b2 + 1) * 256, :].rearrange(
                        "(t p) c -> p t c", p=128))
            x_tiles[b2] = t

        load_x(0, split=True)

        # identity + PE warm-up: ~40 dummy transposes while the DMAs land so
        # the HAM clock gate flips to 2.4 GHz before the real GEMMs start
        id_f = const.tile([128, 128], F32)
        make_identity(nc, id_f[:])
        id_b = const.tile([128, 128], BF16)
        nc.vector.tensor_copy(id_b[:], id_f[:])
        for _ in range(10):
            warm_t = tp_ps.tile([128, 6, 128], BF16, tag="tp")
            for cc in range(6):
                nc.tensor.transpose(warm_t[:, cc, :], id_b[:], id_b[:])

        if NB > 1:
            load_x(1, split=True)

        # resident qkv weights (fp8, pre-scaled + DoubleRow-interleaved on host)
        wq_t = wp.tile([128, 3, 2, C], FP8)
        wk_t = wp.tile([128, 3, 2, C], FP8)
        wv_t = wp.tile([128, 3, 2, C], FP8)
        for src, dst in ((wk_in, wk_t), (wv_in, wv_t), (wq_in, wq_t)):
            nc.sync.dma_start(out=dst[:], in_=src[:])
        wcp_t = wcp_pool.tile([128, 3, 2, C], FP8)
        nc.sync.dma_start(out=wcp_t[:], in_=wcp_in[:])

        # ------- constants / biases (issued after the big DMAs) -------------
        eps_t = const.tile([128, 1], F32)
        nc.vector.memset(eps_t[:], EPS)
        ones_row = const.tile([1, 512], BF16)
        nc.vector.memset(ones_row[:], 1.0)
        # additive pre-exp causal masks (applied via id-stationary matmuls
        # into the score accumulation): slot 0 = -30 above the diagonal,
        # slot 1 = -30 everywhere
        negm = const.tile([128, 2, 128], BF16)
        nc.vector.memset(negm[:, 0, :], 0.0)
        nc.vector.memset(negm[:, 1, :], -30.0)
        nc.gpsimd.affine_select(
            out=negm[:, 0, :],
            in_=negm[:, 0, :],
            compare_op=mybir.AluOpType.is_ge,
            fill=-30.0,
            base=0,
            pattern=[[1, 128]],
            channel_multiplier=-1,
        )
        # per-partition bias tiles [128, 6] (column j = head-pair j)
        bq_sb = const.tile([128, 6], F32)
        bk_sb = const.tile([128, 6], F32)
        for src, dst in ((bq_in, bq_sb), (bk_in, bk_sb)):
            nc.sync.dma_start(out=dst[:], in_=src[:].rearrange("(j p) -> p j", p=128))
        bfc_sb = const.tile([128, 24], F32)
        nc.sync.dma_start(out=bfc_sb[:], in_=bfc_in[:].rearrange("(f p) -> p f", p=128))
        if use_bias:
            # bias rows for ones-matmul adds (bf16, partition 0)
            brow = const.tile([1, 3, C], BF16)
            nc.sync.dma_start(out=brow[:, 0, :], in_=bv_in[:][None, :])
            nc.sync.dma_start(out=brow[:, 1, :], in_=bcp_in[:][None, :])
            nc.sync.dma_start(out=brow[:, 2, :], in_=bpj_in[:][None, :])
            bv_row = brow[:, 0, :]
            bcp_row = brow[:, 1, :]
            bpj_row = brow[:, 2, :]
        nc.vector.memset(v_sb[:, :, :, 64], 1.0)     # softmax-denominator ones

        # software-pipelined: iteration b2 does LN/transpose/copy for block b2
        # and the K/V/Q GEMMs + evictions for block b2-1, so no engine ever
        # waits on a same-block cross-engine handoff
        for b2 in range(NB + 1):
          if b2 < NB:
            if b2 + 2 < NB:
                load_x(b2 + 2)
            x2_t = x_tiles.pop(b2)
            for tt in range(2):
                ti = b2 * 2 + tt
                x_t = x2_t[:, tt, :]
                xg = x_t.rearrange("p (g d) -> p g d", g=3)
                stats = ln_pool.tile([128, 3, 6], F32, tag="st")
                for g in range(3):
                    nc.vector.bn_stats(out=stats[:, g, :], in_=xg[:, g, :])
                mv = ln_pool.tile([128, 2], F32, tag="mv")
                nc.vector.bn_aggr(out=mv[:], in_=stats[:])
                rstd = ln_pool.tile([128, 1], F32, tag="rstd")
                nc.scalar.activation(
                    out=rstd[:], in_=mv[:, 1:2],
                    func=AF.Sqrt, bias=eps_t[:], scale=1.0,
                )
                nc.vector.reciprocal(out=rstd[:], in_=rstd[:])
                h_t = ln_pool.tile([128, C], BF16, tag="h")
                nc.vector.tensor_scalar(
                    out=h_t[:], in0=x_t,
                    scalar1=mv[:, 0:1], scalar2=rstd[:],
                    op0=mybir.AluOpType.subtract, op1=mybir.AluOpType.mult,
                )
                pt = tp_ps.tile([128, 6, 128], BF16, tag="tp")
                for c in range(6):
                    nc.tensor.transpose(pt[:, c, :], h_t[:, c * 128:(c + 1) * 128], id_b[:])
                # PSUM->SBUF (+ fp8 cast) on the Scalar engine: the Vector
                # engine is the stage-A bottleneck, ACT has headroom
                nc.scalar.activation(
                    out=hT_sb[:, :, ti * 128:(ti + 1) * 128], in_=pt[:],
                    func=AF.Copy)
          if b2 >= 1:
            p = b2 - 1
            tb = p * 256
            if p >= 1:
                warm_t = tp_ps.tile([128, 6, 128], BF16, tag="tp")
                for cc in range(4):
                    nc.tensor.transpose(warm_t[:, cc, :], id_b[:], id_b[:])
            # K GEMM: first block at 256 (starts right after tile 0), then
            # 512-token blocks, plus a 256 tail when NB is even
            if p == 0 or p % 2 == 0 or p == NB - 1:
                if p == 0:
                    kb, kw = 0, 256
                elif p % 2 == 0:
                    kb, kw = tb - 256, 512
                else:
                    kb, kw = tb, 256
                for jj in range(6):
                    pk = kq_ps.tile([128, 512], F32, tag="pk")
                    for g in range(3):
                        nc.tensor.matmul(
                            pk[:, 0:kw], wk_t[:, g, :, jj * 128:(jj + 1) * 128],
                            hT_sb[:, 2 * g:2 * g + 2, kb:kb + kw],
                            start=(g == 0), stop=(g == 2), perf_mode=DR,
                        )
                    nc.scalar.activation(
                        out=kT_sb[:, jj, kb:kb + kw], in_=pk[:, 0:kw],
                        func=AF.Identity, bias=bk_sb[:, jj:jj + 1], scale=1.0 / SKV)
            # V GEMM for the two 128-token chunks of this block; one wide
            # PSUM tile so the eviction is a single (cheap) instruction,
            # and it rides the Vector engine (ACT is the stage-A bottleneck)
            for u in range(2):
                ti = p * 2 + u
                for half in range(2):
                    pv = v_ps.tile([128, 384], F32, tag="pv")
                    for g in range(3):
                        nc.tensor.matmul(
                            pv[:],
                            hT_sb[:, 2 * g:2 * g + 2, ti * 128:(ti + 1) * 128],
                            wv_t[:, g, :, half * 384:(half + 1) * 384],
                            start=(g == 0), stop=(g == 2 and not use_bias),
                            perf_mode=DR,
                        )
                    if use_bias:
                        nc.tensor.matmul(
                            pv[:], ones_row[:, 0:128],
                            bv_row[:, half * 384:(half + 1) * 384],
                            start=False, stop=True,
                        )
                    nc.vector.tensor_scalar(
                        out=v_sb[:, ti, half * 6:(half + 1) * 6, 0:64],
                        in0=pv[:].rearrange("p (h d) -> p h d", d=64),
                        scalar1=1.0 / SKV, scalar2=None,
                        op0=mybir.AluOpType.mult,
                    )
            # Q GEMM when this tile is an own strip
            if p in (sA, sB):
                qoff = 0 if p == sA else 256
                for jj in range(6):
                    pq = kq_ps.tile([128, 512], F32, tag="pk")
                    for g in range(3):
                        nc.tensor.matmul(
                            pq[:, 0:256], wq_t[:, g, :, jj * 128:(jj + 1) * 128],
                            hT_sb[:, 2 * g:2 * g + 2, tb:tb + 256],
                            start=(g == 0), stop=(g == 2), perf_mode=DR,
                        )
                    nc.scalar.activation(
                        out=qT_sb[:, jj, qoff:qoff + 256], in_=pq[:, 0:256],
                        func=AF.Identity, bias=bq_sb[:, jj:jj + 1], scale=1.0 / SQ)

        sA_scope.close()   # frees x stream, wq/wk/wv, hT, stage-A PSUM

        # =================== stage B: attention (head pairs) ================
        # c_proj is interleaved per head pair: its partial products accumulate
        # into x1_sb (pre-loaded with the residual x rows) via Vector adds.
        sCP = ExitStack()
        cpb_ps = sCP.enter_context(tc.tile_pool(name="cpb_ps", bufs=2, space="PSUM"))
        sB_scope = ExitStack()
        att_pool = sB_scope.enter_context(tc.tile_pool(name="att", bufs=5))
        nrm_pool = sB_scope.enter_context(tc.tile_pool(name="nrm", bufs=3))
        att_ps = sB_scope.enter_context(tc.tile_pool(name="att_ps", bufs=2, space="PSUM"))
        yt_ps = sB_scope.enter_context(tc.tile_pool(name="yt_ps", bufs=1, space="PSUM"))

        own_rows = (sA * 256, sA * 256 + 128, sB * 256, sB * 256 + 128)
        x1_sb = acts.tile([128, 4, C], F32)
        for m in range(4):
            nc.sync.dma_start(
                out=x1_sb[:, m, :], in_=x_in[own_rows[m]:own_rows[m] + 128, :])

        # dummy exp: prefetches the exp ACT table set under the tail of the
        # stage-A GEMMs instead of stalling the first attention chunk
        scr0 = nrm_pool.tile([128, 1], F32, tag="scr")
        nc.scalar.activation(out=scr0[:], in_=eps_t[:], func=AF.Exp)

        def emit_cproj(rj, ms=range(4)):
            # c_proj partial for head pairs (2rj, 2rj+1), fp8 DoubleRow,
            # accumulated into x1_sb on DVE. Issued late (mid-pair 2rj+2)
            # so the in-order PE stream never waits on a fresh normalize.
            for m in ms:
                for half in range(2):
                    cp = cpb_ps.tile([128, 384], F32, tag="cp")
                    nc.tensor.matmul(
                        cp[:], yT_sb[:, 2 * rj:2 * rj + 2, m * 128:(m + 1) * 128],
                        wcp_t[:, rj, :, half * 384:(half + 1) * 384],
                        start=True, stop=(not use_bias) or rj > 0,
                        perf_mode=DR,
                    )
                    if use_bias and rj == 0:
                        nc.tensor.matmul(
                            cp[:], ones_row[:, 0:128],
                            bcp_row[:, half * 384:(half + 1) * 384],
                            start=False, stop=True,
                        )
                    nc.vector.tensor_add(
                        x1_sb[:, m, half * 384:(half + 1) * 384],
                        x1_sb[:, m, half * 384:(half + 1) * 384],
                        cp[:],
                    )

        n_sh = 2 * (sA + 1)    # kt chunks attended by both strips
        n_all = 2 * (sB + 1)   # kt chunks attended by strip B
        for jj in range(6):
            kT_A = kT_sb[0:64, jj, :]
            kT_B = kT_sb[64:128, jj, :]
            qT_A = qT_sb[0:64, jj, :]
            qT_B = qT_sb[64:128, jj, :]
            yt_A = yt_ps.tile([65, 512], F32, tag="ytA")
            yt_B = yt_ps.tile([65, 512], F32, tag="ytB")
            pending = None

            def issue_av(p):
                # p: list of (kc, at_A_slice, at_B_slice, qs, ww)
                for kc, atA, atB, qs, ww in p:
                    nc.tensor.matmul(
                        yt_A[0:65, qs:qs + ww], v_sb[:, kc, 2 * jj, 0:65],
                        atA, start=(kc == 0), stop=(kc == n_all - 1),
                        skip_group_check=True,
                    )
                    nc.tensor.matmul(
                        yt_B[0:65, qs:qs + ww], v_sb[:, kc, 2 * jj + 1, 0:65],
                        atB, start=(kc == 0), stop=(kc == n_all - 1),
                        skip_group_check=True,
                    )

            def diag_items(kc, col, s):
                # additive mask ops (col, negm-slot) for chunk kc of strip s
                # whose 256-wide q region starts at column col
                if kc == 2 * s:
                    return [(col, 0)]
                if kc == 2 * s + 1:
                    return [(col, 1), (col + 128, 0)]
                return []

            # shared chunks: one chunk per pa tile (q width 512, both strips)
            for kc in range(n_sh):
                mm = diag_items(kc, 0, sA) + diag_items(kc, 256, sB)
                masked = len(mm) > 0
                pa = att_ps.tile([128, 2, 512], F32, tag="pa")
                nc.tensor.matmul(
                    pa[:, 0, :], kT_A[:, kc * 128:(kc + 1) * 128],
                    qT_A[:, 0:512], start=True, stop=not masked,
                    skip_group_check=masked,
                )
                nc.tensor.matmul(
                    pa[:, 1, :], kT_B[:, kc * 128:(kc + 1) * 128],
                    qT_B[:, 0:512], start=True, stop=not masked,
                    skip_group_check=masked,
                )
                for u in range(2):
                    for i, (cs, wh) in enumerate(mm):
                        nc.tensor.matmul(
                            pa[:, u, cs:cs + 128], id_b[:], negm[:, wh, :],
                            start=False, stop=(i == len(mm) - 1),
                            skip_group_check=True,
                        )
                at = att_pool.tile([128, 2, 512], BF16, tag="at")
                nc.scalar.activation(out=at[:], in_=pa[:], func=AF.Exp)
                if pending is not None:
                    issue_av(pending)
                pending = [(kc, at[:, 0, :], at[:, 1, :], 0, 512)]
            # earlier pairs' c_proj, slotted mid-pair into the PE stream
            if jj in (2, 4):
                emit_cproj(jj // 2 - 1)
            # non-shared chunks: pairs of chunks per pa tile (strip B only)
            for kp in range((n_all - n_sh) // 2):
                kc0 = n_sh + 2 * kp
                mm = diag_items(kc0, 0, sB) + diag_items(kc0 + 1, 256, sB)
                masked = len(mm) > 0
                pa = att_ps.tile([128, 2, 512], F32, tag="pa")
                for u in range(2):
                    nc.tensor.matmul(
                        pa[:, 0, u * 256:(u + 1) * 256],
                        kT_A[:, (kc0 + u) * 128:(kc0 + u + 1) * 128],
                        qT_A[:, 256:512],
                        start=(u == 0 or not masked), stop=not masked,
                        skip_group_check=masked,
                    )
                    nc.tensor.matmul(
                        pa[:, 1, u * 256:(u + 1) * 256],
                        kT_B[:, (kc0 + u) * 128:(kc0 + u + 1) * 128],
                        qT_B[:, 256:512],
                        start=(u == 0 or not masked), stop=not masked,
                        skip_group_check=masked,
                    )
                for u in range(2):
                    for i, (cs, wh) in enumerate(mm):
                        nc.tensor.matmul(
                            pa[:, u, cs:cs + 128], id_b[:], negm[:, wh, :],
                            start=False, stop=(i == len(mm) - 1),
                            skip_group_check=True,
                        )
                at = att_pool.tile([128, 2, 512], BF16, tag="at")
                nc.scalar.activation(out=at[:], in_=pa[:], func=AF.Exp)
                if pending is not None:
                    issue_av(pending)
                pending = [
                    (kc0, at[:, 0, 0:256], at[:, 1, 0:256], 256, 256),
                    (kc0 + 1, at[:, 0, 256:512], at[:, 1, 256:512], 256, 256),
                ]
            issue_av(pending)
            # stage yt out of PSUM fast (frees the yt banks for the next pair),
            # then normalize from the SBUF copy
            ytc = nrm_pool.tile([64, 2, 512], F32, tag="ytc")
            sume = nrm_pool.tile([1, 2, 512], F32, tag="sume")
            nc.vector.tensor_copy(sume[:, 0, :], yt_A[64:65, :])
            nc.vector.tensor_copy(ytc[:, 0, :], yt_A[0:64, :])
            nc.vector.tensor_copy(sume[:, 1, :], yt_B[64:65, :])
            nc.vector.tensor_copy(ytc[:, 1, :], yt_B[0:64, :])
            for u, po in ((0, 0), (1, 64)):
                bcast = nrm_pool.tile([64, 512], F32, tag="bcast")
                nc.gpsimd.partition_broadcast(bcast[:], sume[:, u, :])
                nc.vector.reciprocal_approx_fast(out=bcast[:], in_=bcast[:])
                nc.vector.tensor_mul(
                    yT_sb[po:po + 64, jj, :], ytc[0:64, u, :], bcast[:],
                )
            if jj == 5:
                # dummy sqrt: prefetches the sqrt ACT table set while the PE
                # runs c_proj, instead of stalling LN2's critical chain
                scr = nrm_pool.tile([128, 1], F32, tag="scr")
                nc.scalar.activation(out=scr[:], in_=eps_t[:], func=AF.Sqrt)
        for _ in range(2):
            wcp_warm = cpb_ps.tile([128, 384], F32, tag="cp")
            for cc in range(3):
                nc.tensor.transpose(wcp_warm[:, cc * 128:(cc + 1) * 128], id_f[:], id_f[:])
        emit_cproj(2)
        sB_scope.close()
        sAB.close()  # free kT/v/qT before the MLP stages

        # =================== stage C: c_proj, LN2, MLP ======================
        with ExitStack() as sC:
            act46 = sC.enter_context(tc.tile_pool(name="act46", bufs=1))
            ln2_pool = sC.enter_context(tc.tile_pool(name="ln2", bufs=3))
            stream_pool = sC.enter_context(tc.tile_pool(name="stream", bufs=3))
            out_pool = sC.enter_context(tc.tile_pool(name="outp", bufs=3))

            h2T_sb = act46.tile([128, 6, 512], BF16)
            gT_sb = act46.tile([128, 24, 512], BF16)

            # ---- LN2 + transpose (x1_sb already holds x + c_proj(y)) ----
            s4 = ExitStack()
            tp2_ps = s4.enter_context(tc.tile_pool(name="tp2_ps", bufs=2, space="PSUM"))
            for _ in range(2):
                # dummy transposes bridge the PE gap so the HAM clock gate
                # stays at 2.4 GHz into the MLP
                warm2 = tp2_ps.tile([128, 6, 128], BF16, tag="tp2")
                for cc in range(6):
                    nc.tensor.transpose(warm2[:, cc, :], id_b[:], id_b[:])
            for m in range(4):
                # LN2
                x1g = x1_sb[:, m, :].rearrange("p (g d) -> p g d", g=3)
                stats = ln2_pool.tile([128, 3, 6], F32, tag="st2")
                for g in range(3):
                    nc.vector.bn_stats(out=stats[:, g, :], in_=x1g[:, g, :])
                mv = ln2_pool.tile([128, 2], F32, tag="mv2")
                nc.vector.bn_aggr(out=mv[:], in_=stats[:])
                rstd = ln2_pool.tile([128, 1], F32, tag="rstd2")
                nc.scalar.activation(
                    out=rstd[:], in_=mv[:, 1:2],
                    func=AF.Sqrt, bias=eps_t[:], scale=1.0,
                )
                nc.vector.reciprocal(out=rstd[:], in_=rstd[:])
                # -mu*rstd so the LN apply can ride the Scalar engine
                nmr = ln2_pool.tile([128, 1], F32, tag="nmr")
                nc.vector.tensor_scalar(
                    out=nmr[:], in0=mv[:, 0:1],
                    scalar1=rstd[:], scalar2=-1.0,
                    op0=mybir.AluOpType.mult, op1=mybir.AluOpType.mult,
                )
                h2 = ln2_pool.tile([128, C], BF16, tag="h2")
                nc.scalar.activation(
                    out=h2[:], in_=x1_sb[:, m, :],
                    func=AF.Identity, bias=nmr[:], scale=rstd[:],
                )
                pt = tp2_ps.tile([128, 6, 128], BF16, tag="tp2")
                for c in range(6):
                    nc.tensor.transpose(pt[:, c, :], h2[:, c * 128:(c + 1) * 128], id_b[:])
                nc.scalar.activation(
                    out=h2T_sb[:, :, m * 128:(m + 1) * 128], in_=pt[:],
                    func=AF.Copy)
                warm2 = tp2_ps.tile([128, 6, 128], BF16, tag="tp2")
                for cc in range(6):
                    nc.tensor.transpose(warm2[:, cc, :], id_b[:], id_b[:])

            s4.close()
            sCP.close()
            # ---- fc + gelu (wfc streamed 2 f-tiles at a time) ----
            s5 = ExitStack()
            pf_ps = s5.enter_context(tc.tile_pool(name="pf_ps", bufs=3, space="PSUM"))
            for fp in range(12):
                wfc_t = stream_pool.tile([128, 6, 256], BF16, tag="wfc")
                nc.sync.dma_start(
                    out=wfc_t[:],
                    in_=wfc_in[:, fp * 256:(fp + 1) * 256].rearrange(
                        "(c k) n -> k c n", k=128),
                )
                for fi in range(2):
                    f = fp * 2 + fi
                    pf = pf_ps.tile([128, 512], F32, tag="pf")
                    for c in range(6):
                        nc.tensor.matmul(
                            pf[:], wfc_t[:, c, fi * 128:(fi + 1) * 128],
                            h2T_sb[:, c, :],
                            start=(c == 0), stop=(c == 5),
                        )
                    nc.scalar.activation(
                        out=gT_sb[:, f, :], in_=pf[:],
                        func=AF.Gelu_apprx_tanh,
                        bias=bfc_sb[:, f:f + 1], scale=1.0,
                    )

            s5.close()
            # ---- proj + residual + store (one wpj pass) ----
            s6 = ExitStack()
            pj_ps = s6.enter_context(tc.tile_pool(name="pj_ps", bufs=1, space="PSUM"))
            pj = []
            for i in range(8):
                pj_i = pj_ps.tile([128, 384], F32, tag=f"pj{i}")
                pj.append(pj_i)
            for f in range(24):
                wpj_t = stream_pool.tile([128, C], BF16, tag="wpj")
                nc.sync.dma_start(out=wpj_t[:], in_=wpj_in[f * 128:(f + 1) * 128, :])
                for m in range(4):
                    for half in range(2):
                        nc.tensor.matmul(
                            pj[m * 2 + half][:],
                            gT_sb[:, f, m * 128:(m + 1) * 128],
                            wpj_t[:, half * 384:(half + 1) * 384],
                            start=(f == 0), stop=(f == 23 and not use_bias),
                        )
                    if f == 23:
                        # evict + store m as soon as its accumulation closes,
                        # overlapping the remaining m's final matmuls
                        if use_bias:
                            for half in range(2):
                                nc.tensor.matmul(
                                    pj[m * 2 + half][:], ones_row[:, 0:128],
                                    bpj_row[:, half * 384:(half + 1) * 384],
                                    start=False, stop=True,
                                )
                        o_t = out_pool.tile([128, C], F32, tag="o")
                        for half in range(2):
                            nc.vector.tensor_add(
                                o_t[:, half * 384:(half + 1) * 384],
                                pj[m * 2 + half][:],
                                x1_sb[:, m, half * 384:(half + 1) * 384],
                            )
                        nc.sync.dma_start(
                            out=out_dram[m * 128:(m + 1) * 128, :], in_=o_t[:])
            s6.close()


# ---------------------------------------------------------------------------
# Runner
# ---------------------------------------------------------------------------
def _make_runner(nc):
    partition_name = nc.partition_id_tensor.name if nc.partition_id_tensor else None
    in_names, out_names, out_avals, zero_outs = [], [], [], []
    for alloc in nc.m.functions[0].allocations:
        if not isinstance(alloc, mybir.MemoryLocationSet):
            continue
        name = alloc.memorylocations[0].name
        if alloc.kind == "ExternalInput":
            if name != partition_name:
                in_names.append(name)
        elif alloc.kind == "ExternalOutput":
            out_names.append(name)
            shape = tuple(alloc.tensor_shape)
            dtype = mybir.dt.np(alloc.dtype)
            out_avals.append(jax.core.ShapedArray(shape, dtype))
            zero_outs.append(np.zeros(shape, dtype))
    n_params = len(in_names)
    all_names = list(in_names) + list(out_names)
    if partition_name is not None:
        all_names.append(partition_name)

    def _body(*args):
        operands = list(args)
        if partition_name is not None:
            operands.append(partition_id_tensor())
        outs = _bass_exec_p.bind(
            *operands,
            out_avals=tuple(out_avals),
            in_names=tuple(all_names),
            out_names=tuple(out_names),
            lowering_input_output_aliases=(),
            sim_require_finite=True,
            sim_require_nnan=True,
            nc=nc,
        )
        return tuple(outs)

    donate = tuple(range(n_params, n_params + len(out_names)))
    jitted = jax.jit(_body, donate_argnums=donate, keep_unused=True)
    return jitted, in_names, out_names, zero_outs


@functools.lru_cache(maxsize=None)
def _get_runners(use_bias: bool):
    install_neuronx_cc_hook()
    runners = []
    for r in range(4):
        nc = build_rank_program(r, use_bias)
        runners.append(_make_runner(nc))
    return runners


def _prep_core_inputs(x, ln1_w, ln1_b, c_attn_w, c_attn_b, c_proj_w, c_proj_b,
                      ln2_w, ln2_b, fc_w, fc_b, proj_w, proj_b):
    """Fold LN affines into weights; split qkv; pre-cast weights (fp8/bf16)."""
    import ml_dtypes
    f32 = np.float32
    bf16 = ml_dtypes.bfloat16
    fp8 = ml_dtypes.float8_e4m3  # TRN float8e4-compatible (max +-240)
    wqkv = (ln1_w[:, None] * c_attn_w).astype(f32)
    bqkv = (c_attn_b + ln1_b @ c_attn_w).astype(f32)
    scale = f32(1.0 / np.sqrt(HD))

    def dr_pack(w, s):
        # [768, C] -> [128, 3, 2, C] DoubleRow interleave; clip for TRN e4m3
        w8 = np.clip(w * f32(s), -240.0, 240.0).astype(fp8)
        return np.ascontiguousarray(w8.reshape(3, 2, 128, C).transpose(2, 0, 1, 3))

    shared = {
        "wq": dr_pack(wqkv[:, 0:C] * scale, SQ),
        "wk": dr_pack(wqkv[:, C:2 * C], SKV),
        "wv": dr_pack(wqkv[:, 2 * C:3 * C], SKV),
        "bq": np.ascontiguousarray(bqkv[0:C] * scale),
        "bk": np.ascontiguousarray(bqkv[C:2 * C]),
        "bv": np.ascontiguousarray((bqkv[2 * C:3 * C] * SKV).astype(bf16)),
        "wcp": dr_pack(c_proj_w, 1.0),
        "bcp": np.ascontiguousarray(c_proj_b.astype(bf16)),
        "wfc": np.ascontiguousarray((ln2_w[:, None] * fc_w).astype(bf16)),
        "bfc": np.ascontiguousarray((fc_b + ln2_b @ fc_w).astype(f32)),
        "wpj": np.ascontiguousarray(proj_w.astype(bf16)),
        "bpj": np.ascontiguousarray(proj_b.astype(bf16)),
    }
    return shared


def _dispatch_all(inputs):
    """Dispatch the 8 per-core executions asynchronously; return futures."""
    shared = _prep_core_inputs(**{k: np.asarray(v) for k, v in inputs.items()})
    use_bias = bool(
        np.any(np.asarray(shared["bv"], np.float32))
        or np.any(np.asarray(shared["bcp"], np.float32))
        or np.any(np.asarray(shared["bpj"], np.float32)))
    runners = _get_runners(use_bias)
    devices = jax.devices()
    x = np.asarray(inputs["x"], dtype=np.float32)
    futs = []
    for c in range(8):
        b, r = c // 4, c % 4
        jitted, in_names, out_names, zero_outs = runners[r]
        dev = devices[c]
        per_core = dict(shared)
        per_core["x"] = np.ascontiguousarray(x[b])
        args = [jax.device_put(per_core[n], dev) for n in in_names]
        args += [jax.device_put(z, dev) for z in zero_outs]
        futs.append((c, out_names, jitted(*args)))
    return futs


def kernel(**inputs) -> np.ndarray:
    futs = _dispatch_all(inputs)
    out = np.empty((B, T, C), dtype=np.float32)
    for c, out_names, fut in futs:
        b, r = c // 4, c % 4
        res = np.asarray(fut[out_names.index("out")])
        out[b, 256 * r:256 * r + 256] = res[0:256]
        out[b, 256 * (7 - r):256 * (7 - r) + 256] = res[256:512]
    return out

